# revision 51
# baseline (speedup 1.0000x reference)
"""FastSelfAttention Trainium2 kernel — zero-collective batch-per-core.

Reference computation (B=4, S=4096, D=1024):
    h  = layer_norm(hidden_states, g, b)
    q  = h @ Wq.T ; k = h @ Wk.T ; v = q
    qw = exp((q @ wq_att) / sqrt(D) + mask)
    pq = cumsum(qw * q, S) / cumsum(qw, S)
    mk = pq * k
    kw = exp((mk @ wk_att) / sqrt(D) + mask)
    pk = cumsum(kw * mk, S) / cumsum(kw, S)
    out = pk * v

Sharding: one FULL batch per core (cores 4-7 duplicate batches 0-3), so
there are NO collectives and no cross-core sync — each core's NEFF runs
its own batch start-to-finish.  The previous e-split design spent ~19ms
on silicon stalled around its AllReduce; this one runs ~0.37ms.

Layout is feature-major [e, s]; cumsum runs along the free (s) axis via
DVE tensor_tensor_scan (f32 internal state, bf16 outputs), chained
across s-chunks with carry columns.

LayerNorm folding: xs = (h - mu) * rstd computed in-place from two
partition-broadcast rows (rstd, -mu*rstd); weights are pre-scaled by g
host-side (W' = W * g), and the bias contribution (W @ b, zero for this
problem's ln_b) would ride the PSUM->SBUF copy as a per-partition bias
(use_cbias).  l1 = rstd*(svq.h) + (b.vq/sqrt(D) + mask) with
svq = vqp - colsvq/D folds the mean subtraction into the l1 stationary
host-side.  The 1/den1 row is factored out of the second logit's e-sum
(l2 = rden1 * (wkp . (n1*k)), u2 = (kw*rden1) x nk), so the pooled
query is never materialized.

Per-chunk phases are software-pipelined three deep — front(c+1) [stats
matmuls, LN/logit rows, broadcasts, xs] || mid(c) [projections, pool-1
scans] || pool2(c-1) — so the PE never stalls on the row/scan chains;
q_all has 3 buffers to cover its 3-iteration lifetime.  Engine map: PE
stats+projections+l2, DVE scans+big elementwise muls, Act psum->sbuf
copies + fp8 squares + exp/ln rows (rstd = exp(-0.5*ln(var+eps)),
rden = exp(-ln(den)) — keeps every activation in ONE table set,
enforced by narrowing the act-table map so a single LoadActFuncSet is
hoisted to entry), Pool partition-broadcasts.  The first h chunks are
DMA-prefetched ahead of the 4MB of weights; the last chunk runs per-e
so the tail drains early.
"""

import numpy as np
import ml_dtypes

import concourse.bass as bass
import concourse.bacc as bacc
import concourse.mybir as mybir
import concourse.tile as tile
from concourse.bass_utils import run_bass_kernel_spmd

dt = mybir.dt
AF = mybir.ActivationFunctionType
OP = mybir.AluOpType

B, S, D = 4, 4096, 1024
NC = 8               # cores
SC = 512             # s-chunk
NSC = S // SC        # 8 s-chunks
ND = D // 128        # 8 d-chunks
NE = D // 128        # 8 e-chunks (full feature range per core)
INV_SQRT_D = 1.0 / np.sqrt(np.float32(D))
EPS = 1e-5

_prog_cache = {}


def _build_program(use_cbias=False, repeat=1):
    key = ("bpc", use_cbias, repeat)
    if key in _prog_cache:
        return _prog_cache[key]

    # All activation funcs used below (Copy/Square/Exp/Ln/Identity) live in
    # the natural_log_exp_and_others table set; restricting the table list
    # lets the act-table pass hoist a single load to program entry instead
    # of thrashing between per-function first-match sets.
    _orig_tables = bacc.get_activation_tables
    _tgt_set = "natural_log_exp_and_others"
    _my_funcs = {AF.Exp, AF.Ln, AF.Copy, AF.Square, AF.Identity}

    def _one_table(arch):
        tabs = _orig_tables(arch)
        if _tgt_set not in tabs or not _my_funcs <= set(tabs[_tgt_set]):
            return tabs
        # act_func_set_id is positional: keep every entry in order, but
        # make _tgt_set the only set claiming the functions we use.
        return {k: (v if k == _tgt_set else set(v) - _my_funcs)
                for k, v in tabs.items()}

    bacc.get_activation_tables = _one_table

    nc = bacc.Bacc("TRN2", num_devices=NC)
    f32, bf16, f8 = dt.float32, dt.bfloat16, dt.float8e4

    # ---- external I/O ----
    hb = nc.dram_tensor("hb", [D, S], bf16, kind="ExternalInput")
    wqT = nc.dram_tensor("wqT", [D, D], bf16, kind="ExternalInput")
    wkT = nc.dram_tensor("wkT", [D, D], bf16, kind="ExternalInput")
    svq_in = nc.dram_tensor("svq", [128, ND], bf16, kind="ExternalInput")
    ccq_in = nc.dram_tensor("ccq", [128, NE], f32, kind="ExternalInput")
    cck_in = nc.dram_tensor("cck", [128, NE], f32, kind="ExternalInput")
    wkp_in = nc.dram_tensor("wkp", [128, NE], bf16, kind="ExternalInput")
    mrow1_in = nc.dram_tensor("mrow1", [1, S], f32, kind="ExternalInput")
    mrow2_in = nc.dram_tensor("mrow2", [1, S], f32, kind="ExternalInput")

    outT = nc.dram_tensor("outT", [D, S], bf16, kind="ExternalOutput")

    with tile.TileContext(nc) as tc:
        with (
            tc.tile_pool(name="const", bufs=1) as cpool,
            tc.tile_pool(name="persist", bufs=1) as ppool,
            tc.tile_pool(name="rows", bufs=1) as rows,
            tc.tile_pool(name="bcast", bufs=2) as bc,
            tc.tile_pool(name="rd1", bufs=3) as rd1,
            tc.tile_pool(name="ht", bufs=2) as wht,
            tc.tile_pool(name="sq", bufs=1) as wsq,
            tc.tile_pool(name="qa", bufs=3) as wqa,
            tc.tile_pool(name="ka", bufs=2) as wqk,
            tc.tile_pool(name="scr", bufs=2) as wscr,
            tc.tile_pool(name="nsc", bufs=2) as wnsc,
            tc.tile_pool(name="mk", bufs=2) as wmk,
            tc.tile_pool(name="o", bufs=1) as wo,
            tc.tile_pool(name="psA", bufs=4, space="PSUM") as psA,
            tc.tile_pool(name="psR", bufs=1, space="PSUM") as psR,
            tc.tile_pool(name="psL2", bufs=1, space="PSUM") as psL2,
        ):
            # ---- resident constants ----
            # svq + first two h chunks BEFORE the 4MB of weights: DMA
            # transfers serialize, chunk-0 stats need svq+ht first, and
            # the weights are only needed ~25us in (first proj)
            svq_t = cpool.tile([128, ND], bf16, tag="svq")
            nc.sync.dma_start(out=svq_t[:], in_=svq_in[:])
            _pref = {}
            for _cc in (0, 1):
                _t = wht.tile([128, ND, SC], bf16, tag="ht")
                nc.sync.dma_start(
                    out=_t[:],
                    in_=hb.rearrange("(a p) s -> p a s",
                                     p=128)[:, :, _cc * SC:(_cc + 1) * SC])
                _pref[_cc] = _t

            ccq_t = cpool.tile([128, NE], f32, tag="ccq")
            cck_t = cpool.tile([128, NE], f32, tag="cck")
            nc.sync.dma_start(out=ccq_t[:], in_=ccq_in[:])
            nc.sync.dma_start(out=cck_t[:], in_=cck_in[:])

            wkp_t = cpool.tile([128, NE], bf16, tag="wkp")
            nc.sync.dma_start(out=wkp_t[:], in_=wkp_in[:])

            wq_t = cpool.tile([128, ND, D], bf16, tag="wq")
            wk_t = cpool.tile([128, ND, D], bf16, tag="wk")
            nc.sync.dma_start(
                out=wq_t[:], in_=wqT.rearrange("(a p) e -> p a e", p=128))
            nc.sync.dma_start(
                out=wk_t[:], in_=wkT.rearrange("(a p) e -> p a e", p=128))

            ones8 = cpool.tile([128, 2, 16], f8, tag="ones8")
            nc.vector.memset(ones8[:], 1.0)
            eps_t = cpool.tile([1, 1], f32, tag="eps")
            nc.vector.memset(eps_t[:], EPS)

            # ---- persistent carries ----
            carry_q = ppool.tile([128, NE], bf16, tag="carry_q")
            carry_k = ppool.tile([128, NE], bf16, tag="carry_k")
            carry_d = ppool.tile([1, 2], f32, tag="carry_d")
            nc.vector.memset(carry_q[:], 0.0)
            nc.vector.memset(carry_k[:], 0.0)
            nc.vector.memset(carry_d[:], 0.0)

            hh = ND // 2
            fstate = {}
            state = {}

            def front(cc):
                s0 = cc * SC
                if cc in _pref:
                    ht = _pref.pop(cc)
                else:
                    ht = wht.tile([128, ND, SC], bf16, tag="ht")
                    nc.sync.dma_start(
                        out=ht[:],
                        in_=hb.rearrange("(a p) s -> p a s",
                                         p=128)[:, :, s0:s0 + SC])

                h8 = wsq.tile([128, ND, SC], f8, tag="h8")
                if cc < 2:
                    # startup: DVE is idle, keep Act free for the row chain
                    nc.vector.tensor_copy(h8[:], ht[:])
                else:
                    nc.scalar.activation(h8[:], ht[:], AF.Copy)
                sq = wsq.tile([128, ND, SC], f8, tag="sq")
                nc.scalar.activation(sq[:], ht[:], AF.Square)

                # stats rows: l1p first (needs only ht, so PE starts
                # before the Act fp8 copies land), then st/sxx fp8 DR
                l1p_ps = psR.tile([1, SC], f32, tag="l1p")
                for d in range(ND):
                    nc.tensor.matmul(l1p_ps[:], svq_t[:, d:d + 1], ht[:, d, :],
                                     start=(d == 0), stop=(d == ND - 1))
                st_ps = psR.tile([1, SC], f32, tag="st")
                for d in range(0, ND, 2):
                    nc.tensor.matmul(
                        st_ps[:], ones8[:, :, 0:1], h8[:, d:d + 2, :],
                        start=(d == 0), stop=(d == ND - 2),
                        perf_mode=mybir.MatmulPerfMode.DoubleRow)
                sxx_ps = psR.tile([1, SC], f32, tag="sxx")
                for d in range(0, ND, 2):
                    nc.tensor.matmul(
                        sxx_ps[:], ones8[:, :, 0:1], sq[:, d:d + 2, :],
                        start=(d == 0), stop=(d == ND - 2),
                        perf_mode=mybir.MatmulPerfMode.DoubleRow)

                # LN rows
                musq = rows.tile([1, SC], f32, tag="musq")
                nc.scalar.activation(musq[:], st_ps[:], AF.Square,
                                     scale=1.0 / D)
                var = rows.tile([1, SC], f32, tag="var")
                nc.vector.scalar_tensor_tensor(
                    var[:], sxx_ps[:], 1.0 / D, musq[:],
                    OP.mult, OP.subtract)
                lnv = rows.tile([1, SC], f32, tag="lnv")
                nc.scalar.activation(lnv[:], var[:], AF.Ln, bias=eps_t[:])
                rstd = rows.tile([1, SC], f32, tag="rstd")
                nc.scalar.activation(rstd[:], lnv[:], AF.Exp, scale=-0.5)
                rstd_h = rows.tile([1, SC], bf16, tag="rstd_h")
                nc.scalar.activation(rstd_h[:], lnv[:], AF.Exp, scale=-0.5)
                nmur = rows.tile([1, SC], bf16, tag="nmur")
                nc.vector.scalar_tensor_tensor(
                    nmur[:], st_ps[:], -1.0 / D, rstd[:],
                    OP.mult, OP.mult)

                # l1 -> qw
                l1f = rows.tile([1, SC], f32, tag="l1f")
                nc.vector.tensor_mul(l1f[:], l1p_ps[:], rstd[:])
                m1s = rows.tile([1, SC], f32, tag="m1s")
                nc.sync.dma_start(out=m1s[:], in_=mrow1_in[:, s0:s0 + SC])
                l1b = rows.tile([1, SC], f32, tag="l1b")
                nc.vector.tensor_add(l1b[:], l1f[:], m1s[:])
                qw = rows.tile([1, SC], bf16, tag="qw")
                nc.scalar.activation(qw[:], l1b[:], AF.Exp)

                # den1 scan + rden1 = exp(-ln(den1))
                den1 = rows.tile([1, SC], f32, tag="den1")
                init1 = 0.0 if cc == 0 else carry_d[:, 0:1]
                nc.vector.tensor_tensor_scan(
                    den1[:], qw[:], qw[:], init1, OP.add, OP.bypass)
                nc.vector.tensor_copy(carry_d[:, 0:1], den1[:, SC - 1:SC])
                lnd1 = rows.tile([1, SC], f32, tag="lnd1")
                nc.scalar.activation(lnd1[:], den1[:], AF.Ln)
                rden1h = rd1.tile([1, SC], bf16, tag="rden1h")
                nc.scalar.activation(rden1h[:], lnd1[:], AF.Exp, scale=-1.0)

                # broadcasts (Pool)
                rstd_b = bc.tile([128, SC], bf16, tag="rstd_b")
                nc.gpsimd.partition_broadcast(rstd_b[:], rstd_h[:])
                nmur_b = bc.tile([128, SC], bf16, tag="nmur_b")
                nc.gpsimd.partition_broadcast(nmur_b[:], nmur[:])
                qb = bc.tile([128, SC], bf16, tag="qb")
                nc.gpsimd.partition_broadcast(qb[:], qw[:])

                # xs = ht*rstd + nmur  (in-place, halves)
                for p0 in range(2):
                    sl = slice(p0 * hh, (p0 + 1) * hh)
                    nc.vector.tensor_mul(
                        ht[:, sl, :], ht[:, sl, :],
                        rstd_b[:].unsqueeze(1).broadcast_to([128, hh, SC]))
                for p0 in range(2):
                    sl = slice(p0 * hh, (p0 + 1) * hh)
                    nc.vector.tensor_add(
                        ht[:, sl, :], ht[:, sl, :],
                        nmur_b[:].unsqueeze(1).broadcast_to([128, hh, SC]))
                fstate[cc] = (ht, qb, rden1h)

            def mid(cc):
                xs, qb, rden1h = fstate.pop(cc)

                # projections
                q_all = wqa.tile([128, NE, SC], bf16, tag="q_all")
                k_all = wqk.tile([128, NE, SC], bf16, tag="k_all")
                for e in range(NE):
                    es = slice(e * 128, (e + 1) * 128)
                    q_ps = psA.tile([128, SC], f32, tag="proj")
                    for d in range(ND):
                        nc.tensor.matmul(
                            q_ps[:], wq_t[:, d, es], xs[:, d, :],
                            start=(d == 0), stop=(d == ND - 1))
                    if use_cbias:
                        nc.scalar.activation(q_all[:, e, :], q_ps[:],
                                             AF.Identity,
                                             bias=ccq_t[:, e:e + 1])
                    else:
                        nc.scalar.copy(q_all[:, e, :], q_ps[:])
                for e in range(NE):
                    es = slice(e * 128, (e + 1) * 128)
                    k_ps = psA.tile([128, SC], f32, tag="proj")
                    for d in range(ND):
                        nc.tensor.matmul(
                            k_ps[:], wk_t[:, d, es], xs[:, d, :],
                            start=(d == 0), stop=(d == ND - 1))
                    if use_cbias:
                        nc.scalar.activation(k_all[:, e, :], k_ps[:],
                                             AF.Identity,
                                             bias=cck_t[:, e:e + 1])
                    else:
                        nc.scalar.copy(k_all[:, e, :], k_ps[:])

                # u1 = qb * q ; n1 scans ; k' = db*k ; mk = n1*k'
                u1 = wscr.tile([128, NE, SC], bf16, tag="u")
                if cc != NSC - 1:
                    nc.vector.tensor_mul(
                        u1[:], q_all[:],
                        qb[:].unsqueeze(1).broadcast_to([128, NE, SC]))
                n1 = wnsc.tile([128, NE, SC], bf16, tag="n")
                for e in range(NE):
                    if cc == NSC - 1:
                        nc.vector.tensor_mul(u1[:, e, :], q_all[:, e, :],
                                             qb[:])
                    init = 0.0 if cc == 0 else carry_q[:, e:e + 1]
                    nc.vector.tensor_tensor_scan(
                        n1[:, e, :], u1[:, e, :], u1[:, e, :], init,
                        OP.add, OP.bypass)
                    nc.vector.tensor_copy(carry_q[:, e:e + 1],
                                          n1[:, e, SC - 1:SC])
                nk = wmk.tile([128, NE, SC], bf16, tag="mk")
                if cc == NSC - 1:
                    # last chunk: per-e so l2 matmuls can start early
                    for e in range(NE):
                        nc.vector.tensor_mul(nk[:, e, :], n1[:, e, :],
                                             k_all[:, e, :])
                else:
                    nc.vector.tensor_mul(nk[:], n1[:], k_all[:])

                state[cc] = (q_all, nk, rden1h, cc * SC)

            def pool2(cc):
                q_all, nk, rden1h, s0 = state.pop(cc)
                last = (s0 == (NSC - 1) * SC)
                outv = outT.rearrange("(a p) s -> p a s", p=128)
                m2s = rows.tile([1, SC], f32, tag="m2s")
                nc.sync.dma_start(out=m2s[:], in_=mrow2_in[:, s0:s0 + SC])
                lg2 = rows.tile([1, SC], f32, tag="lg2")
                l2r = rows.tile([1, SC], f32, tag="l2r")
                kw = rows.tile([1, SC], bf16, tag="kw")
                r2w = rows.tile([1, SC], bf16, tag="r2w")
                den2 = rows.tile([1, SC], f32, tag="den2")
                lnd2 = rows.tile([1, SC], f32, tag="lnd2")
                rden2h = rows.tile([1, SC], bf16, tag="rden2h")
                kb = bc.tile([128, SC], bf16, tag="kb")
                d2b = bc.tile([128, SC], bf16, tag="d2b")
                u2 = wscr.tile([128, NE, SC], bf16, tag="u")
                n2 = wnsc.tile([128, NE, SC], bf16, tag="n")
                o1 = wscr.tile([128, NE, SC], bf16, tag="u")
                o = wo.tile([128, NE, SC], bf16, tag="o")

                # the last chunk runs in two s-halves so its serial tail
                # (rows -> scans -> out) is half as deep
                subs = [(0, SC)]
                for (lo, hi) in subs:
                    sl = slice(lo, hi)
                    # l2 row (den1 scaling factored out of the e-sum)
                    l2_ps = psL2.tile([1, SC], f32, tag="l2")
                    for e in range(NE):
                        nc.tensor.matmul(l2_ps[:, sl], wkp_t[:, e:e + 1],
                                         nk[:, e, sl],
                                         start=(e == 0), stop=(e == NE - 1))
                    nc.vector.tensor_mul(l2r[:, sl], l2_ps[:, sl],
                                         rden1h[:, sl])
                    nc.vector.tensor_add(lg2[:, sl], l2r[:, sl], m2s[:, sl])
                    nc.scalar.activation(kw[:, sl], lg2[:, sl], AF.Exp)

                    init2 = 0.0 if s0 + lo == 0 else carry_d[:, 1:2]
                    nc.vector.tensor_tensor_scan(
                        den2[:, sl], kw[:, sl], kw[:, sl], init2,
                        OP.add, OP.bypass)
                    nc.vector.tensor_copy(carry_d[:, 1:2],
                                          den2[:, hi - 1:hi])
                    nc.scalar.activation(lnd2[:, sl], den2[:, sl], AF.Ln)
                    nc.scalar.activation(rden2h[:, sl], lnd2[:, sl],
                                         AF.Exp, scale=-1.0)

                    nc.vector.tensor_mul(r2w[:, sl], kw[:, sl],
                                         rden1h[:, sl])
                    nc.gpsimd.partition_broadcast(kb[:, sl], r2w[:, sl])
                    nc.gpsimd.partition_broadcast(d2b[:, sl],
                                                  rden2h[:, sl])

                    if last:
                        for e in range(NE):
                            nc.vector.tensor_mul(u2[:, e, sl], nk[:, e, sl],
                                                 kb[:, sl])
                            init = (0.0 if s0 + lo == 0
                                    else carry_k[:, e:e + 1])
                            nc.vector.tensor_tensor_scan(
                                n2[:, e, sl], u2[:, e, sl], u2[:, e, sl],
                                init, OP.add, OP.bypass)
                            nc.vector.tensor_copy(carry_k[:, e:e + 1],
                                                  n2[:, e, hi - 1:hi])
                            nc.vector.tensor_mul(o1[:, e, sl],
                                                 n2[:, e, sl],
                                                 q_all[:, e, sl])
                            nc.vector.tensor_mul(o[:, e, sl], o1[:, e, sl],
                                                 d2b[:, sl])
                            nc.sync.dma_start(
                                out=outv[:, e:e + 1, s0 + lo:s0 + hi],
                                in_=o[:, e:e + 1, sl])
                    else:
                        nc.vector.tensor_mul(
                            u2[:], nk[:],
                            kb[:].unsqueeze(1).broadcast_to([128, NE, SC]))
                        for e in range(NE):
                            init = 0.0 if s0 == 0 else carry_k[:, e:e + 1]
                            nc.vector.tensor_tensor_scan(
                                n2[:, e, :], u2[:, e, :], u2[:, e, :], init,
                                OP.add, OP.bypass)
                            nc.vector.tensor_copy(carry_k[:, e:e + 1],
                                                  n2[:, e, SC - 1:SC])
                        # o = (n2*q) * rden2b — n2*q first so q_all frees
                        nc.vector.tensor_mul(o1[:], n2[:], q_all[:])
                        nc.vector.tensor_mul(
                            o[:], o1[:],
                            d2b[:].unsqueeze(1).broadcast_to([128, NE, SC]))
                        nc.sync.dma_start(
                            out=outv[:, :, s0:s0 + SC],
                            in_=o[:])

            for _r in range(repeat):
                for cc in range(NSC + 2):
                    if cc < NSC:
                        front(cc)
                    if 1 <= cc <= NSC:
                        mid(cc - 1)
                    if cc >= 2:
                        pool2(cc - 2)

    try:
        nc.finalize()
    finally:
        bacc.get_activation_tables = _orig_tables
    _prog_cache[key] = nc
    return nc


def _host_prep(hidden_states, attention_mask, Wq, wq_att, Wk, wk_att, ln_g, ln_b):
    """Build the 8 per-core input maps (batch b = core % 4)."""
    f4 = np.float32
    g = np.asarray(ln_g, f4)
    bb = np.asarray(ln_b, f4)
    Wq = np.asarray(Wq, f4)
    Wk = np.asarray(Wk, f4)
    wq_att = np.asarray(wq_att, f4)[:, 0]
    wk_att = np.asarray(wk_att, f4)[:, 0]
    h = np.asarray(hidden_states, f4)
    am = np.asarray(attention_mask, f4)

    Wqp = Wq * g[None, :]           # [e,d]
    Wkp = Wk * g[None, :]
    wqT_full = np.ascontiguousarray(Wqp.T)   # [d,e]
    wkT_full = np.ascontiguousarray(Wkp.T)
    cq = Wq @ bb                    # [e] (zero when ln_b == 0)
    ck = Wk @ bb

    vq = Wq.T @ wq_att              # [d]
    vqp = (g * vq) * INV_SQRT_D     # [d]
    cvq = float(bb @ vq) * INV_SQRT_D
    colsvq = float(vqp.sum())
    wkp_full = (wk_att * INV_SQRT_D).astype(f4)

    maskb = (1.0 - am) * -10000.0   # [B,S]

    def bf(a):
        return np.ascontiguousarray(
            np.asarray(a, f4).astype(ml_dtypes.bfloat16))

    # stationary [svq | ones]: svq folds the l1 mean subtraction
    svq = np.ascontiguousarray((vqp - colsvq / D).reshape(ND, 128).T)  # [128, ND]

    ccq = np.ascontiguousarray(cq.reshape(NE, 128).T)       # [128, NE]
    cck = np.ascontiguousarray(ck.reshape(NE, 128).T)
    wkp = bf(wkp_full.reshape(NE, 128).T)                   # [128, NE]

    in_maps = []
    for core in range(NC):
        b = core % B
        in_maps.append({
            "hb": bf(h[b].T),
            "wqT": bf(wqT_full),
            "wkT": bf(wkT_full),
            "svq": bf(svq),
            "ccq": ccq.astype(f4),
            "cck": cck.astype(f4),
            "wkp": wkp,
            "mrow1": np.ascontiguousarray((maskb[b] + cvq).reshape(1, S)),
            "mrow2": np.ascontiguousarray(maskb[b].reshape(1, S)),
        })
    return in_maps, bool(np.any(cq != 0.0) or np.any(ck != 0.0))


def kernel(**inputs):
    import time as _time
    in_maps, use_cbias = _host_prep(**inputs)
    nc = _build_program(use_cbias=use_cbias)
    res = None
    last = None
    for _attempt in range(3):
        try:
            res = run_bass_kernel_spmd(nc, in_maps, core_ids=list(range(NC)))
            break
        except Exception as e:  # transient first-exec device faults self-heal
            last = e
            _time.sleep(3)
    if res is None:
        raise last
    out = np.empty((B, S, D), np.float32)
    for b in range(B):
        out[b] = res.results[b]["outT"].astype(np.float32).T
    return out


# revision 52
# speedup vs baseline: 1.0938x; 1.0938x over previous
"""FastSelfAttention Trainium2 kernel — zero-collective batch-per-core.

Reference computation (B=4, S=4096, D=1024):
    h  = layer_norm(hidden_states, g, b)
    q  = h @ Wq.T ; k = h @ Wk.T ; v = q
    qw = exp((q @ wq_att) / sqrt(D) + mask)
    pq = cumsum(qw * q, S) / cumsum(qw, S)
    mk = pq * k
    kw = exp((mk @ wk_att) / sqrt(D) + mask)
    pk = cumsum(kw * mk, S) / cumsum(kw, S)
    out = pk * v

Sharding: one FULL batch per core (cores 4-7 duplicate batches 0-3), so
there are NO collectives and no cross-core sync — each core's NEFF runs
its own batch start-to-finish.  The previous e-split design spent ~19ms
on silicon stalled around its AllReduce; this one runs ~0.37ms.

Layout is feature-major [e, s]; cumsum runs along the free (s) axis via
DVE tensor_tensor_scan (f32 internal state, bf16 outputs), chained
across s-chunks with carry columns.

LayerNorm folding: xs = (h - mu) * rstd computed in-place from two
partition-broadcast rows (rstd, -mu*rstd); weights are pre-scaled by g
host-side (W' = W * g), and the bias contribution (W @ b, zero for this
problem's ln_b) would ride the PSUM->SBUF copy as a per-partition bias
(use_cbias).  l1 = rstd*(svq.h) + (b.vq/sqrt(D) + mask) with
svq = vqp - colsvq/D folds the mean subtraction into the l1 stationary
host-side.  The 1/den1 row is factored out of the second logit's e-sum
(l2 = rden1 * (wkp . (n1*k)), u2 = (kw*rden1) x nk), so the pooled
query is never materialized.

Per-chunk phases are software-pipelined three deep — front(c+1) [stats
matmuls, LN/logit rows, broadcasts, xs] || mid(c) [projections, pool-1
scans] || pool2(c-1) — so the PE never stalls on the row/scan chains;
q_all has 3 buffers to cover its 3-iteration lifetime.  Engine map: PE
stats+projections+l2, DVE scans+big elementwise muls, Act psum->sbuf
copies + fp8 squares + exp/ln rows (rstd = exp(-0.5*ln(var+eps)),
rden = exp(-ln(den)) — keeps every activation in ONE table set,
enforced by narrowing the act-table map so a single LoadActFuncSet is
hoisted to entry), Pool partition-broadcasts.  The first h chunks are
DMA-prefetched ahead of the 4MB of weights; the last chunk runs per-e
so the tail drains early.
"""

import numpy as np
import ml_dtypes

import concourse.bass as bass
import concourse.bacc as bacc
import concourse.mybir as mybir
import concourse.tile as tile
from concourse.bass_utils import run_bass_kernel_spmd

dt = mybir.dt
AF = mybir.ActivationFunctionType
OP = mybir.AluOpType

B, S, D = 4, 4096, 1024
NC = 8               # cores
SC = 512             # s-chunk
NSC = S // SC        # 8 s-chunks
ND = D // 128        # 8 d-chunks
NE = D // 128        # 8 e-chunks (full feature range per core)
INV_SQRT_D = 1.0 / np.sqrt(np.float32(D))
EPS = 1e-5

_prog_cache = {}


def _build_program(use_cbias=False, repeat=1):
    key = ("bpc", use_cbias, repeat)
    if key in _prog_cache:
        return _prog_cache[key]

    # All activation funcs used below (Copy/Square/Exp/Ln/Identity) live in
    # the natural_log_exp_and_others table set; restricting the table list
    # lets the act-table pass hoist a single load to program entry instead
    # of thrashing between per-function first-match sets.
    _orig_tables = bacc.get_activation_tables
    _tgt_set = "natural_log_exp_and_others"
    _my_funcs = {AF.Exp, AF.Ln, AF.Copy, AF.Square, AF.Identity}

    def _one_table(arch):
        tabs = _orig_tables(arch)
        if _tgt_set not in tabs or not _my_funcs <= set(tabs[_tgt_set]):
            return tabs
        # act_func_set_id is positional: keep every entry in order, but
        # make _tgt_set the only set claiming the functions we use.
        return {k: (v if k == _tgt_set else set(v) - _my_funcs)
                for k, v in tabs.items()}

    bacc.get_activation_tables = _one_table

    nc = bacc.Bacc("TRN2", num_devices=NC)
    f32, bf16, f8 = dt.float32, dt.bfloat16, dt.float8e4

    # ---- external I/O ----
    hb = nc.dram_tensor("hb", [D, S], bf16, kind="ExternalInput")
    wqT = nc.dram_tensor("wqT", [D, D], bf16, kind="ExternalInput")
    wkT = nc.dram_tensor("wkT", [D, D], bf16, kind="ExternalInput")
    svq_in = nc.dram_tensor("svq", [128, ND], bf16, kind="ExternalInput")
    ccq_in = nc.dram_tensor("ccq", [128, NE], f32, kind="ExternalInput")
    cck_in = nc.dram_tensor("cck", [128, NE], f32, kind="ExternalInput")
    wkp_in = nc.dram_tensor("wkp", [128, NE], bf16, kind="ExternalInput")
    mrow1_in = nc.dram_tensor("mrow1", [1, S], f32, kind="ExternalInput")
    mrow2_in = nc.dram_tensor("mrow2", [1, S], f32, kind="ExternalInput")

    outT = nc.dram_tensor("outT", [D, S], bf16, kind="ExternalOutput")

    with tile.TileContext(nc) as tc:
        with (
            tc.tile_pool(name="const", bufs=1) as cpool,
            tc.tile_pool(name="persist", bufs=1) as ppool,
            tc.tile_pool(name="rows", bufs=1) as rows,
            tc.tile_pool(name="bcast", bufs=2) as bc,
            tc.tile_pool(name="rd1", bufs=3) as rd1,
            tc.tile_pool(name="ht", bufs=2) as wht,
            tc.tile_pool(name="sq", bufs=1) as wsq,
            tc.tile_pool(name="qa", bufs=3) as wqa,
            tc.tile_pool(name="ka", bufs=2) as wqk,
            tc.tile_pool(name="scr", bufs=2) as wscr,
            tc.tile_pool(name="nsc", bufs=2) as wnsc,
            tc.tile_pool(name="mk", bufs=2) as wmk,
            tc.tile_pool(name="o", bufs=1) as wo,
            tc.tile_pool(name="psA", bufs=4, space="PSUM") as psA,
            tc.tile_pool(name="psR", bufs=1, space="PSUM") as psR,
            tc.tile_pool(name="psL2", bufs=1, space="PSUM") as psL2,
        ):
            # ---- resident constants ----
            # svq + first two h chunks BEFORE the 4MB of weights: DMA
            # transfers serialize, chunk-0 stats need svq+ht first, and
            # the weights are only needed ~25us in (first proj)
            svq_t = cpool.tile([128, ND], bf16, tag="svq")
            nc.sync.dma_start(out=svq_t[:], in_=svq_in[:])
            _pref = {}
            for _cc in (0, 1):
                _t = wht.tile([128, ND, SC], bf16, tag="ht")
                nc.sync.dma_start(
                    out=_t[:],
                    in_=hb.rearrange("(a p) s -> p a s",
                                     p=128)[:, :, _cc * SC:(_cc + 1) * SC])
                _pref[_cc] = _t

            ccq_t = cpool.tile([128, NE], f32, tag="ccq")
            cck_t = cpool.tile([128, NE], f32, tag="cck")
            nc.sync.dma_start(out=ccq_t[:], in_=ccq_in[:])
            nc.sync.dma_start(out=cck_t[:], in_=cck_in[:])

            wkp_t = cpool.tile([128, NE], bf16, tag="wkp")
            nc.sync.dma_start(out=wkp_t[:], in_=wkp_in[:])

            wq_t = cpool.tile([128, ND, D], bf16, tag="wq")
            wk_t = cpool.tile([128, ND, D], bf16, tag="wk")
            nc.sync.dma_start(
                out=wq_t[:], in_=wqT.rearrange("(a p) e -> p a e", p=128))
            nc.sync.dma_start(
                out=wk_t[:], in_=wkT.rearrange("(a p) e -> p a e", p=128))

            ones8 = cpool.tile([128, 2, 16], f8, tag="ones8")
            nc.vector.memset(ones8[:], 1.0)
            eps_t = cpool.tile([1, 1], f32, tag="eps")
            nc.vector.memset(eps_t[:], EPS)

            # ---- persistent carries ----
            carry_q = ppool.tile([128, NE], bf16, tag="carry_q")
            carry_k = ppool.tile([128, NE], bf16, tag="carry_k")
            carry_d = ppool.tile([1, 2], f32, tag="carry_d")
            nc.vector.memset(carry_q[:], 0.0)
            nc.vector.memset(carry_k[:], 0.0)
            nc.vector.memset(carry_d[:], 0.0)

            hh = ND // 2
            fstate = {}
            state = {}

            def front(cc):
                s0 = cc * SC
                if cc in _pref:
                    ht = _pref.pop(cc)
                else:
                    ht = wht.tile([128, ND, SC], bf16, tag="ht")
                    nc.sync.dma_start(
                        out=ht[:],
                        in_=hb.rearrange("(a p) s -> p a s",
                                         p=128)[:, :, s0:s0 + SC])

                h8 = wsq.tile([128, ND, SC], f8, tag="h8")
                if cc < 2:
                    # startup: DVE is idle, keep Act free for the row chain
                    nc.vector.tensor_copy(h8[:], ht[:])
                else:
                    nc.scalar.activation(h8[:], ht[:], AF.Copy)
                sq = wsq.tile([128, ND, SC], f8, tag="sq")
                nc.scalar.activation(sq[:], ht[:], AF.Square)

                # stats rows: st first, then l1p, then sxx
                st_ps = psR.tile([1, SC], f32, tag="st")
                for d in range(0, ND, 2):
                    nc.tensor.matmul(
                        st_ps[:], ones8[:, :, 0:1], h8[:, d:d + 2, :],
                        start=(d == 0), stop=(d == ND - 2),
                        perf_mode=mybir.MatmulPerfMode.DoubleRow)
                l1p_ps = psR.tile([1, SC], f32, tag="l1p")
                for d in range(ND):
                    nc.tensor.matmul(l1p_ps[:], svq_t[:, d:d + 1], ht[:, d, :],
                                     start=(d == 0), stop=(d == ND - 1))
                sxx_ps = psR.tile([1, SC], f32, tag="sxx")
                for d in range(0, ND, 2):
                    nc.tensor.matmul(
                        sxx_ps[:], ones8[:, :, 0:1], sq[:, d:d + 2, :],
                        start=(d == 0), stop=(d == ND - 2),
                        perf_mode=mybir.MatmulPerfMode.DoubleRow)

                # LN rows
                musq = rows.tile([1, SC], f32, tag="musq")
                nc.scalar.activation(musq[:], st_ps[:], AF.Square,
                                     scale=1.0 / D)
                var = rows.tile([1, SC], f32, tag="var")
                nc.vector.scalar_tensor_tensor(
                    var[:], sxx_ps[:], 1.0 / D, musq[:],
                    OP.mult, OP.subtract)
                lnv = rows.tile([1, SC], f32, tag="lnv")
                nc.scalar.activation(lnv[:], var[:], AF.Ln, bias=eps_t[:])
                rstd = rows.tile([1, SC], f32, tag="rstd")
                nc.scalar.activation(rstd[:], lnv[:], AF.Exp, scale=-0.5)
                rstd_h = rows.tile([1, SC], bf16, tag="rstd_h")
                nc.scalar.activation(rstd_h[:], lnv[:], AF.Exp, scale=-0.5)
                nmur = rows.tile([1, SC], bf16, tag="nmur")
                nc.vector.scalar_tensor_tensor(
                    nmur[:], st_ps[:], -1.0 / D, rstd[:],
                    OP.mult, OP.mult)

                # l1 -> qw
                l1f = rows.tile([1, SC], f32, tag="l1f")
                nc.vector.tensor_mul(l1f[:], l1p_ps[:], rstd[:])
                m1s = rows.tile([1, SC], f32, tag="m1s")
                nc.sync.dma_start(out=m1s[:], in_=mrow1_in[:, s0:s0 + SC])
                l1b = rows.tile([1, SC], f32, tag="l1b")
                nc.vector.tensor_add(l1b[:], l1f[:], m1s[:])
                qw = rows.tile([1, SC], bf16, tag="qw")
                nc.scalar.activation(qw[:], l1b[:], AF.Exp)

                # den1 scan + rden1 = exp(-ln(den1))
                den1 = rows.tile([1, SC], f32, tag="den1")
                init1 = 0.0 if cc == 0 else carry_d[:, 0:1]
                nc.vector.tensor_tensor_scan(
                    den1[:], qw[:], qw[:], init1, OP.add, OP.bypass)
                nc.vector.tensor_copy(carry_d[:, 0:1], den1[:, SC - 1:SC])
                lnd1 = rows.tile([1, SC], f32, tag="lnd1")
                nc.scalar.activation(lnd1[:], den1[:], AF.Ln)
                rden1h = rd1.tile([1, SC], bf16, tag="rden1h")
                nc.scalar.activation(rden1h[:], lnd1[:], AF.Exp, scale=-1.0)

                # broadcasts (Pool)
                rstd_b = bc.tile([128, SC], bf16, tag="rstd_b")
                nc.gpsimd.partition_broadcast(rstd_b[:], rstd_h[:])
                nmur_b = bc.tile([128, SC], bf16, tag="nmur_b")
                nc.gpsimd.partition_broadcast(nmur_b[:], nmur[:])
                qb = bc.tile([128, SC], bf16, tag="qb")
                nc.gpsimd.partition_broadcast(qb[:], qw[:])

                # xs = ht*rstd + nmur  (in-place, halves)
                for p0 in range(2):
                    sl = slice(p0 * hh, (p0 + 1) * hh)
                    nc.vector.tensor_mul(
                        ht[:, sl, :], ht[:, sl, :],
                        rstd_b[:].unsqueeze(1).broadcast_to([128, hh, SC]))
                for p0 in range(2):
                    sl = slice(p0 * hh, (p0 + 1) * hh)
                    nc.vector.tensor_add(
                        ht[:, sl, :], ht[:, sl, :],
                        nmur_b[:].unsqueeze(1).broadcast_to([128, hh, SC]))
                fstate[cc] = (ht, qb, rden1h)

            def mid(cc):
                xs, qb, rden1h = fstate.pop(cc)

                # projections
                q_all = wqa.tile([128, NE, SC], bf16, tag="q_all")
                k_all = wqk.tile([128, NE, SC], bf16, tag="k_all")
                for e in range(NE):
                    es = slice(e * 128, (e + 1) * 128)
                    q_ps = psA.tile([128, SC], f32, tag="proj")
                    for d in range(ND):
                        nc.tensor.matmul(
                            q_ps[:], wq_t[:, d, es], xs[:, d, :],
                            start=(d == 0), stop=(d == ND - 1))
                    if use_cbias:
                        nc.scalar.activation(q_all[:, e, :], q_ps[:],
                                             AF.Identity,
                                             bias=ccq_t[:, e:e + 1])
                    else:
                        nc.scalar.copy(q_all[:, e, :], q_ps[:])
                for e in range(NE):
                    es = slice(e * 128, (e + 1) * 128)
                    k_ps = psA.tile([128, SC], f32, tag="proj")
                    for d in range(ND):
                        nc.tensor.matmul(
                            k_ps[:], wk_t[:, d, es], xs[:, d, :],
                            start=(d == 0), stop=(d == ND - 1))
                    if use_cbias:
                        nc.scalar.activation(k_all[:, e, :], k_ps[:],
                                             AF.Identity,
                                             bias=cck_t[:, e:e + 1])
                    else:
                        nc.scalar.copy(k_all[:, e, :], k_ps[:])

                # u1 = qb * q ; n1 scans ; k' = db*k ; mk = n1*k'
                u1 = wscr.tile([128, NE, SC], bf16, tag="u")
                if cc != NSC - 1:
                    nc.vector.tensor_mul(
                        u1[:], q_all[:],
                        qb[:].unsqueeze(1).broadcast_to([128, NE, SC]))
                n1 = wnsc.tile([128, NE, SC], bf16, tag="n")
                for e in range(NE):
                    if cc == NSC - 1:
                        nc.vector.tensor_mul(u1[:, e, :], q_all[:, e, :],
                                             qb[:])
                    init = 0.0 if cc == 0 else carry_q[:, e:e + 1]
                    nc.vector.tensor_tensor_scan(
                        n1[:, e, :], u1[:, e, :], u1[:, e, :], init,
                        OP.add, OP.bypass)
                    nc.vector.tensor_copy(carry_q[:, e:e + 1],
                                          n1[:, e, SC - 1:SC])
                nk = wmk.tile([128, NE, SC], bf16, tag="mk")
                if cc == NSC - 1:
                    # last chunk: per-e so l2 matmuls can start early
                    for e in range(NE):
                        nc.vector.tensor_mul(nk[:, e, :], n1[:, e, :],
                                             k_all[:, e, :])
                else:
                    nc.vector.tensor_mul(nk[:], n1[:], k_all[:])

                state[cc] = (q_all, nk, rden1h, cc * SC)

            def pool2(cc):
                q_all, nk, rden1h, s0 = state.pop(cc)
                last = (s0 == (NSC - 1) * SC)
                outv = outT.rearrange("(a p) s -> p a s", p=128)
                m2s = rows.tile([1, SC], f32, tag="m2s")
                nc.sync.dma_start(out=m2s[:], in_=mrow2_in[:, s0:s0 + SC])
                lg2 = rows.tile([1, SC], f32, tag="lg2")
                l2r = rows.tile([1, SC], f32, tag="l2r")
                kw = rows.tile([1, SC], bf16, tag="kw")
                r2w = rows.tile([1, SC], bf16, tag="r2w")
                den2 = rows.tile([1, SC], f32, tag="den2")
                lnd2 = rows.tile([1, SC], f32, tag="lnd2")
                rden2h = rows.tile([1, SC], bf16, tag="rden2h")
                kb = bc.tile([128, SC], bf16, tag="kb")
                d2b = bc.tile([128, SC], bf16, tag="d2b")
                u2 = wscr.tile([128, NE, SC], bf16, tag="u")
                n2 = wnsc.tile([128, NE, SC], bf16, tag="n")
                o1 = wscr.tile([128, NE, SC], bf16, tag="u")
                o = wo.tile([128, NE, SC], bf16, tag="o")

                # the last chunk runs in two s-halves so its serial tail
                # (rows -> scans -> out) is half as deep
                subs = [(0, SC)]
                for (lo, hi) in subs:
                    sl = slice(lo, hi)
                    # l2 row (den1 scaling factored out of the e-sum)
                    l2_ps = psL2.tile([1, SC], f32, tag="l2")
                    for e in range(NE):
                        nc.tensor.matmul(l2_ps[:, sl], wkp_t[:, e:e + 1],
                                         nk[:, e, sl],
                                         start=(e == 0), stop=(e == NE - 1))
                    nc.vector.tensor_mul(l2r[:, sl], l2_ps[:, sl],
                                         rden1h[:, sl])
                    nc.vector.tensor_add(lg2[:, sl], l2r[:, sl], m2s[:, sl])
                    nc.scalar.activation(kw[:, sl], lg2[:, sl], AF.Exp)

                    init2 = 0.0 if s0 + lo == 0 else carry_d[:, 1:2]
                    nc.vector.tensor_tensor_scan(
                        den2[:, sl], kw[:, sl], kw[:, sl], init2,
                        OP.add, OP.bypass)
                    nc.vector.tensor_copy(carry_d[:, 1:2],
                                          den2[:, hi - 1:hi])
                    nc.scalar.activation(lnd2[:, sl], den2[:, sl], AF.Ln)
                    nc.scalar.activation(rden2h[:, sl], lnd2[:, sl],
                                         AF.Exp, scale=-1.0)

                    nc.vector.tensor_mul(r2w[:, sl], kw[:, sl],
                                         rden1h[:, sl])
                    nc.gpsimd.partition_broadcast(kb[:, sl], r2w[:, sl])
                    nc.gpsimd.partition_broadcast(d2b[:, sl],
                                                  rden2h[:, sl])

                    if last:
                        for e in range(NE):
                            nc.vector.tensor_mul(u2[:, e, sl], nk[:, e, sl],
                                                 kb[:, sl])
                            init = (0.0 if s0 + lo == 0
                                    else carry_k[:, e:e + 1])
                            nc.vector.tensor_tensor_scan(
                                n2[:, e, sl], u2[:, e, sl], u2[:, e, sl],
                                init, OP.add, OP.bypass)
                            nc.vector.tensor_copy(carry_k[:, e:e + 1],
                                                  n2[:, e, hi - 1:hi])
                            nc.vector.tensor_mul(o1[:, e, sl],
                                                 n2[:, e, sl],
                                                 q_all[:, e, sl])
                            nc.vector.tensor_mul(o[:, e, sl], o1[:, e, sl],
                                                 d2b[:, sl])
                            nc.sync.dma_start(
                                out=outv[:, e:e + 1, s0 + lo:s0 + hi],
                                in_=o[:, e:e + 1, sl])
                    else:
                        nc.vector.tensor_mul(
                            u2[:], nk[:],
                            kb[:].unsqueeze(1).broadcast_to([128, NE, SC]))
                        for e in range(NE):
                            init = 0.0 if s0 == 0 else carry_k[:, e:e + 1]
                            nc.vector.tensor_tensor_scan(
                                n2[:, e, :], u2[:, e, :], u2[:, e, :], init,
                                OP.add, OP.bypass)
                            nc.vector.tensor_copy(carry_k[:, e:e + 1],
                                                  n2[:, e, SC - 1:SC])
                        # o = (n2*q) * rden2b — n2*q first so q_all frees
                        nc.vector.tensor_mul(o1[:], n2[:], q_all[:])
                        nc.vector.tensor_mul(
                            o[:], o1[:],
                            d2b[:].unsqueeze(1).broadcast_to([128, NE, SC]))
                        nc.sync.dma_start(
                            out=outv[:, :, s0:s0 + SC],
                            in_=o[:])

            for _r in range(repeat):
                for cc in range(NSC + 2):
                    if cc < NSC:
                        front(cc)
                    if 1 <= cc <= NSC:
                        mid(cc - 1)
                    if cc >= 2:
                        pool2(cc - 2)

    try:
        nc.finalize()
    finally:
        bacc.get_activation_tables = _orig_tables
    _prog_cache[key] = nc
    return nc


def _host_prep(hidden_states, attention_mask, Wq, wq_att, Wk, wk_att, ln_g, ln_b):
    """Build the 8 per-core input maps (batch b = core % 4)."""
    f4 = np.float32
    g = np.asarray(ln_g, f4)
    bb = np.asarray(ln_b, f4)
    Wq = np.asarray(Wq, f4)
    Wk = np.asarray(Wk, f4)
    wq_att = np.asarray(wq_att, f4)[:, 0]
    wk_att = np.asarray(wk_att, f4)[:, 0]
    h = np.asarray(hidden_states, f4)
    am = np.asarray(attention_mask, f4)

    Wqp = Wq * g[None, :]           # [e,d]
    Wkp = Wk * g[None, :]
    wqT_full = np.ascontiguousarray(Wqp.T)   # [d,e]
    wkT_full = np.ascontiguousarray(Wkp.T)
    cq = Wq @ bb                    # [e] (zero when ln_b == 0)
    ck = Wk @ bb

    vq = Wq.T @ wq_att              # [d]
    vqp = (g * vq) * INV_SQRT_D     # [d]
    cvq = float(bb @ vq) * INV_SQRT_D
    colsvq = float(vqp.sum())
    wkp_full = (wk_att * INV_SQRT_D).astype(f4)

    maskb = (1.0 - am) * -10000.0   # [B,S]

    def bf(a):
        return np.ascontiguousarray(
            np.asarray(a, f4).astype(ml_dtypes.bfloat16))

    # stationary [svq | ones]: svq folds the l1 mean subtraction
    svq = np.ascontiguousarray((vqp - colsvq / D).reshape(ND, 128).T)  # [128, ND]

    ccq = np.ascontiguousarray(cq.reshape(NE, 128).T)       # [128, NE]
    cck = np.ascontiguousarray(ck.reshape(NE, 128).T)
    wkp = bf(wkp_full.reshape(NE, 128).T)                   # [128, NE]

    in_maps = []
    for core in range(NC):
        b = core % B
        in_maps.append({
            "hb": bf(h[b].T),
            "wqT": bf(wqT_full),
            "wkT": bf(wkT_full),
            "svq": bf(svq),
            "ccq": ccq.astype(f4),
            "cck": cck.astype(f4),
            "wkp": wkp,
            "mrow1": np.ascontiguousarray((maskb[b] + cvq).reshape(1, S)),
            "mrow2": np.ascontiguousarray(maskb[b].reshape(1, S)),
        })
    return in_maps, bool(np.any(cq != 0.0) or np.any(ck != 0.0))


def kernel(**inputs):
    import time as _time
    in_maps, use_cbias = _host_prep(**inputs)
    nc = _build_program(use_cbias=use_cbias)
    res = None
    last = None
    for _attempt in range(3):
        try:
            res = run_bass_kernel_spmd(nc, in_maps, core_ids=list(range(NC)))
            break
        except Exception as e:  # transient first-exec device faults self-heal
            last = e
            _time.sleep(3)
    if res is None:
        raise last
    out = np.empty((B, S, D), np.float32)
    for b in range(B):
        out[b] = res.results[b]["outT"].astype(np.float32).T
    return out
